# revision 1
# baseline (speedup 1.0000x reference)
"""Trainium2 Bass kernel for the channel-attention module (optimized).

Reference computation (B=16, N=4096, C=384, H=8, D=48):
    x_in = x @ conv_w.T + conv_b                      # 1x1 conv == linear
    q    = (x_in @ wq.T + bq)  -> [B,H,D,N]
    k, v = (x_in @ wkv.T + bkv) -> 2x [B,H,D,N]
    attn = softmax((q * N**-0.5) @ k^T, axis=-1)      # [B,H,D,D] (over N!)
    out  = attn @ v                                   # [B,H,D,N]
    out  = out.transpose(0,2,1,3).reshape(B,N,C)      # verbatim torch layout
    y    = out @ wp.T + bp

Strategy: data parallel over B (2 batches/core), conv folded into the
projections on the host.  Per batch, four big GEMM passes:
  A) v-projection (bf16) + Gram G' = [X|1]^T[X|1] in fp8e4 DoubleRow
     (256-token contraction per matmul at 0.5 cyc/col; fp8 feeds only the
     softmax logits, which tolerate it -- rel err 2.3e-3 overall),
  B) small S-chain: T'' = G' wq_ext, S^T = wk_ext^T T''_h per head
     (extended weights [w; b] with the Gram's ones-row/column absorb the
     rank-1 bias terms; S is computed TRANSPOSED via G's symmetry so the
     softmax axis lands on partitions and normalized attn^T comes out
     directly: exp on ACT, column sums via a ones-vector matmul,
     reciprocal on DVE, broadcast via a rank-1 matmul), then attn^T is
     scattered into block-diagonal bd tiles with 10 shift-matrix matmuls
     (built on-chip with affine_select; no DMA scatter),
  C) stage5 AT[c',u] = vT^T bd, with u = 256*di+32*h+t so the awkward
     transpose(0,2,1,3).reshape layout falls out of 128-token flat
     blocks, and stage6 y = AT^T wp + bp read back with stride-3 APs.

Schedule: the two batches are software-pipelined -- batch b's S-chain
interleaves into the next batch's streaming phase so the PE never waits
on the ACT/DVE chain.  The Gram lags the v-projection by 2 windows so
the fp8 loads (issued on the ACT HWDGE queue) have arrival slack.  PE
warmup runs off a gpsimd-memset tile so nothing gates on DMA at
startup, and the first v-projection consumes criticality-ordered
split DMA pieces.  Output is written bf16 (upcast on the host), batched
per 4 token-chunks via gpsimd software-DGE mid-kernel, and streamed as
single chunks on the two HWDGE queues for the last 8 chunks so the
final writeback drains with the compute instead of after it.
"""

import sys
import types
from contextlib import ExitStack

import numpy as np

import concourse.bass as bass
import concourse.tile as tile
from concourse import bacc, mybir
from concourse.bass_utils import run_bass_kernel_spmd
from concourse.masks import make_identity

B, N, C, H, D = 16, 4096, 384, 8, 48
N_CORES = 8
BPC = B // N_CORES          # batches per core
NW = 512                    # token window for the v projection
NWIN = N // NW              # 8 windows
NCHUNK = N // 128           # 32 token chunks of 128
XNP = 512                   # padded row length of the fp8 xn tensor
SCALE = float(N) ** -0.5    # 1/64
F32 = mybir.dt.float32
F32R = mybir.dt.float32r
BF16 = mybir.dt.bfloat16
F8 = mybir.dt.float8e4

# block-diag scatter: (kc, h, off) with off = 48h - 128kc; Sh[dj, c_p] = 1
# iff c_p == dj + off places head h's attn^T rows into bd tile kc.
SCATTER = [
    (0, 0, 0), (0, 1, 48), (0, 2, 96),
    (1, 2, -32), (1, 3, 16), (1, 4, 64), (1, 5, 112),
    (2, 5, -16), (2, 6, 32), (2, 7, 80),
]
# u-column range of bd tile kc actually covered by heads (rest stays 0)
BD_COLS = {0: (0, 144), 1: (96, 288), 2: (240, 384)}


def _install_ntff_hook():
    """The agent image's antenv lacks axon_hooks, so trn_boot's NTFF hook
    registration degrades silently and trace=True would crash.  Recreate the
    module and register the ctypes hook so profiling works."""
    try:
        import antenv

        if "antenv.axon_hooks" in sys.modules:
            return
        mod = types.ModuleType("antenv.axon_hooks")
        mod._hook = None
        mod.set_axon_ntff_profile_hook = lambda h: setattr(mod, "_hook", h)
        mod.get_axon_ntff_profile_hook = lambda: mod._hook
        sys.modules["antenv.axon_hooks"] = mod
        antenv.axon_hooks = mod
        from trn_agent_boot.trn_boot import _ntff_profile_via_ctypes

        mod.set_axon_ntff_profile_hook(
            _ntff_profile_via_ctypes("/opt/axon/libaxon_pjrt.so")
        )
    except Exception:
        pass


def build():
    nc = bacc.Bacc("TRN2", target_bir_lowering=False, debug=False,
                   num_devices=N_CORES)

    xp = nc.declare_dram_parameter("x", [BPC, 128, NWIN, 3, NW], BF16,
                                   isOutput=False)
    xn_p = nc.declare_dram_parameter("xn", [BPC, 128, NCHUNK, XNP], F8,
                                     isOutput=False)
    wq_p = nc.declare_dram_parameter("wqT", [C, C], BF16, isOutput=False)
    wk_p = nc.declare_dram_parameter("wkT", [C, C], BF16, isOutput=False)
    wv_p = nc.declare_dram_parameter("wvT", [C, C], BF16, isOutput=False)
    wp_p = nc.declare_dram_parameter("wpT", [C, C], F32R, isOutput=False)
    bq_p = nc.declare_dram_parameter("bq", [1, C], BF16, isOutput=False)
    bk_p = nc.declare_dram_parameter("bk", [1, C], BF16, isOutput=False)
    bqn_p = nc.declare_dram_parameter("bqN", [1, C], F32, isOutput=False)
    bv_p = nc.declare_dram_parameter("bv", [C], F32, isOutput=False)
    bp_p = nc.declare_dram_parameter("bp", [C], F32, isOutput=False)
    outp = nc.declare_dram_parameter("out", [BPC, N, C], BF16, isOutput=True)
    scr_p = nc.declare_dram_parameter("scr", [128, 1], F32, isOutput=True)

    with tile.TileContext(nc) as tc, ExitStack() as ctx:
        const = ctx.enter_context(tc.tile_pool(name="const", bufs=1))
        xin = ctx.enter_context(tc.tile_pool(name="xin", bufs=4))
        xnp = ctx.enter_context(tc.tile_pool(name="xnp", bufs=7))
        big = ctx.enter_context(tc.tile_pool(name="big", bufs=2))
        big1 = ctx.enter_context(tc.tile_pool(name="big1", bufs=1))
        sm = ctx.enter_context(tc.tile_pool(name="sm", bufs=2))
        yo = ctx.enter_context(tc.tile_pool(name="yo", bufs=6))
        ps_g = ctx.enter_context(tc.tile_pool(name="ps_g", bufs=3, space="PSUM"))
        ps_v = ctx.enter_context(tc.tile_pool(name="ps_v", bufs=2, space="PSUM"))
        ps_c = ctx.enter_context(tc.tile_pool(name="ps_c", bufs=3, space="PSUM"))

        # ---- PE warmup tile first (gpsimd memset; no DMA dependency) ------
        warm = const.tile([128, NW], BF16)
        nc.gpsimd.memset(warm[:], 0.0)

        # ---- first window of batch 0: issue the minimum data the first
        # v-projection needs in strict criticality order, in small pieces, so
        # it isn't gated on the whole prefetch backlog.
        xb0 = xp.ap()[0][:, 0]  # [128, 3, NW] window 0
        xw0 = xin.tile([128, 3, NW], BF16, tag="xw", name="xw0")
        wv_sb = const.tile([128, 3, C], BF16, tag="w_wv")
        wv_ap = wv_p.ap().rearrange("(kc p) o -> p kc o", p=128)
        xnb0 = xn_p.ap()[0]  # [128, NCHUNK, XNP]
        xnw0 = xnp.tile([128, 4, XNP], F8, tag="xnw", name="xnw0")
        nc.scalar.dma_start(xnw0[:, 0:2, :], xnb0[:, 0:2, :])
        nc.scalar.dma_start(xnw0[:, 2:4, :], xnb0[:, 2:4, :])
        for kc in range(3):
            nc.sync.dma_start(xw0[:, kc, 0:NW // 2], xb0[:, kc, 0:NW // 2])
            nc.sync.dma_start(wv_sb[:, kc, :], wv_ap[:, kc, :])
        for kc in range(3):
            nc.sync.dma_start(xw0[:, kc, NW // 2:NW], xb0[:, kc, NW // 2:NW])
        bv_sb = const.tile([128, 3], F32)
        nc.sync.dma_start(bv_sb[:], bv_p.ap().rearrange("(oc p) -> p oc", p=128))

        # ---- small constants built on gpsimd (no DMA) ---------------------
        sh_sb = const.tile([48, len(SCATTER), 128], BF16, tag="sh")
        nc.gpsimd.memset(sh_sb[:], 0.0)
        for i, (kc, h, off) in enumerate(SCATTER):
            nc.gpsimd.affine_select(
                out=sh_sb[:, i, :], in_=sh_sb[:, i, :],
                compare_op=mybir.AluOpType.not_equal,
                fill=1.0, base=off, pattern=[[-1, 128]], channel_multiplier=1,
            )
        id128 = const.tile([128, 128], BF16)
        make_identity(nc, id128[:])
        ones48 = const.tile([48, 1], BF16)
        nc.gpsimd.memset(ones48[:], 1.0)
        ones_r = const.tile([1, 48], F32)
        nc.gpsimd.memset(ones_r[:], 1.0)

        # warmup matmuls ramp the PE p-state while the first DMAs land
        warm_ps = ps_c.tile([128, NW], F32, tag="psc", name="warm_ps")
        for i in range(16):
            nc.tensor.matmul(warm_ps[:], warm[:, 0:128], warm[:],
                             start=(i == 0), stop=(i == 15))
        warm_sb = const.tile([128, 1], F32)
        nc.vector.tensor_copy(warm_sb[:], warm_ps[:, 0:1])
        nc.scalar.dma_start(scr_p.ap()[:, :], warm_sb[:])
        exp_warm = const.tile([1, 1], F32)
        nc.scalar.activation(exp_warm[:], warm_ps[0:1, 0:1],
                             mybir.ActivationFunctionType.Exp,
                             bias=0.0, scale=1.0)

        # ---- deferred weight/bias loads (emitted later in the schedule) ---
        def load_w(param, dtype):
            t = const.tile([128, 3, C], dtype, tag=f"w_{param.name}")
            nc.sync.dma_start(
                t[:], param.ap().rearrange("(kc p) o -> p kc o", p=128))
            return t

        wq_sb = wk_sb = wp_sb = None
        bq_row = bk_row = bqn_row = bp_bc = None

        def load_qk():
            nonlocal wq_sb, wk_sb, bq_row, bk_row, bqn_row
            wq_sb = load_w(wq_p, BF16)
            wk_sb = load_w(wk_p, BF16)
            bq_row = const.tile([1, C], BF16)
            nc.sync.dma_start(bq_row[:], bq_p.ap()[:, :])
            bk_row = const.tile([1, C], BF16)
            nc.sync.dma_start(bk_row[:], bk_p.ap()[:, :])
            bqn_row = const.tile([1, C], F32)
            nc.sync.dma_start(bqn_row[:], bqn_p.ap()[:, :])

        def load_p():
            nonlocal wp_sb, bp_bc
            bp_ap = bp_p.ap()
            bp_bc = const.tile([128, C], F32)
            nc.sync.dma_start(bp_bc[:], bass.AP(
                tensor=bp_ap.tensor, offset=bp_ap.offset,
                ap=[[0, 128], *bp_ap.ap]))
            wp_sb = load_w(wp_p, F32R)

        # ---- per-batch state ----------------------------------------------
        st = [dict() for _ in range(BPC)]

        def v_part(b, w):
            s = st[b]
            if w == 0:
                s["vT"] = big.tile([128, 3, N], BF16, tag="vT", name=f"vT{b}")
                s["xnw"] = {}
            if b == 0 and w == 0:
                xw, xnw = xw0, xnw0
            else:
                xw = xin.tile([128, 3, NW], BF16, tag="xw")
                nc.sync.dma_start(xw[:], xp.ap()[b][:, w])
                xnw = xnp.tile([128, 4, XNP], F8, tag="xnw")
                nc.scalar.dma_start(xnw[:],
                                    xn_p.ap()[b][:, 4 * w:4 * w + 4, :])
            s["xnw"][w] = xnw

            vT = s["vT"]
            halves = [(0, NW)] if not (b == 0 and w == 0) else \
                [(0, NW // 2), (NW // 2, NW)]
            for t0, t1 in halves:
                for oc in range(3):
                    v_ps = ps_v.tile([128, t1 - t0], F32, tag="vps",
                                     name="v_ps")
                    for kc in range(3):
                        nc.tensor.matmul(
                            v_ps[:],
                            wv_sb[:, kc, oc * 128:(oc + 1) * 128],
                            xw[:, kc, t0:t1],
                            start=(kc == 0), stop=(kc == 2),
                        )
                    half = (t1 - t0) // 2
                    wsl_a = slice(w * NW + t0, w * NW + t0 + half)
                    wsl_b = slice(w * NW + t0 + half, w * NW + t1)
                    nc.scalar.activation(
                        vT[:, oc, wsl_a], v_ps[:, 0:half],
                        mybir.ActivationFunctionType.Identity,
                        bias=bv_sb[:, oc:oc + 1], scale=1.0)
                    nc.vector.tensor_scalar_add(
                        vT[:, oc, wsl_b], v_ps[:, half:t1 - t0],
                        bv_sb[:, oc:oc + 1])

        def g_part(b, w):
            # Gram in fp8 DoubleRow: 2 token-chunks (256 contraction) per mm.
            # Runs 2 windows behind the v-projection so the fp8 loads have
            # plenty of arrival slack.
            s = st[b]
            if w == 0:
                s["g_ps"] = [ps_g.tile([128, C + 1], F32, tag="g",
                                       name=f"g{b}_{i}") for i in range(3)]
            xnw = s["xnw"].pop(w)
            for sp in range(2):
                tp = 2 * w + sp
                ksl = slice(2 * sp, 2 * sp + 2)
                for oc in range(3):
                    nc.tensor.matmul(
                        s["g_ps"][oc][:],
                        xnw[:, ksl, oc * 128:(oc + 1) * 128],
                        xnw[:, ksl, 0:C + 1],
                        start=(tp == 0), stop=(tp == 2 * NWIN - 1),
                        perf_mode=mybir.MatmulPerfMode.DoubleRow,
                    )

        GLAG = 5

        def a_step(b, w):
            v_part(b, w)
            if w >= GLAG:
                g_part(b, w - GLAG)

        def a_tail(b):
            for w in range(NWIN - GLAG, NWIN):
                g_part(b, w)

        # ---- S-chain stages (phase B). Batch 1's transient PSUM tiles use
        # the Gram pool (its banks are free by then); batch 0's use ps_c.
        def bpool(b):
            return ps_g if b == 1 else ps_c

        def btag(b):
            return "g" if b == 1 else "psc"

        def b_s1_act(b):
            s = st[b]
            g_sb = sm.tile([128, 3, C + 1], BF16, tag="g_sb", name=f"g_sb{b}")
            s["g_sb"] = g_sb
            for oc in range(2):
                nc.scalar.activation(
                    g_sb[:, oc, :], s["g_ps"][oc][:],
                    mybir.ActivationFunctionType.Identity, bias=0.0, scale=1.0)
            nc.vector.tensor_copy(g_sb[:, 2, :], s["g_ps"][2][:])

        def b_s1_pe(b):
            s = st[b]
            g_sb = s["g_sb"]
            # s^T row via identity matmul (s lives in G\'s ones column)
            st_ps = bpool(b).tile([1, C], F32, tag=btag(b), name="st_ps")
            for kc in range(3):
                nc.tensor.matmul(st_ps[:, kc * 128:(kc + 1) * 128],
                                 g_sb[:, kc, C:C + 1], id128[:],
                                 start=True, stop=True)
            st_row = sm.tile([1, C], BF16, tag="st_row", name="st_row")
            nc.vector.tensor_copy(st_row[:], st_ps[:])
            # T\'\' row C = wq^T s + N bq
            tr_ps = bpool(b).tile([1, C], F32, tag=btag(b), name="tr_ps")
            for kc in range(3):
                nc.tensor.matmul(tr_ps[:], g_sb[:, kc, C:C + 1],
                                 wq_sb[:, kc, :],
                                 start=(kc == 0), stop=(kc == 2))
            t_row = sm.tile([1, C], BF16, tag="t_row", name="t_row")
            nc.vector.tensor_add(t_row[:], tr_ps[:], bqn_row[:])
            s["st_row"], s["t_row"] = st_row, t_row

        def b_s2(b):
            # T\'\' = G\' wq_ext, c1 blocks (G symmetry: lhsT from G rows)
            s = st[b]
            g_sb = s["g_sb"]
            t_sb = sm.tile([128, 3, C], BF16, tag="t_sb", name=f"t_sb{b}")
            s["t_sb"] = t_sb
            for c1 in range(3):
                t_ps = bpool(b).tile([128, C], F32, tag=btag(b), name="t_ps")
                for kc2 in range(3):
                    nc.tensor.matmul(
                        t_ps[:],
                        g_sb[:, kc2, c1 * 128:(c1 + 1) * 128],
                        wq_sb[:, kc2, :],
                        start=(kc2 == 0), stop=False)
                nc.tensor.matmul(
                    t_ps[:], s["st_row"][:, c1 * 128:(c1 + 1) * 128],
                    bq_row[:], start=False, stop=True)
                if c1 < 2:
                    nc.scalar.activation(
                        t_sb[:, c1, :], t_ps[:],
                        mybir.ActivationFunctionType.Identity,
                        bias=0.0, scale=1.0)
                else:
                    nc.vector.tensor_copy(t_sb[:, c1, :], t_ps[:])

        def b_s34(b, hs):
            # S^T_h = wk_ext_h^T T\'\'_h (softmax axis lands on partitions)
            s = st[b]
            if "s_ps" not in s:
                s["s_ps"] = bpool(b).tile([48, H, 48], F32, tag=btag(b),
                                          name="s_ps")
            s_ps, t_sb = s["s_ps"], s["t_sb"]
            for h in hs:
                hsl = slice(48 * h, 48 * (h + 1))
                for kc1 in range(3):
                    nc.tensor.matmul(
                        s_ps[:, h, :], wk_sb[:, kc1, hsl], t_sb[:, kc1, hsl],
                        start=(kc1 == 0), stop=False)
                nc.tensor.matmul(s_ps[:, h, :], bk_row[:, hsl],
                                 s["t_row"][:, hsl], start=False, stop=True)

        def b_s5a(b):
            s = st[b]
            p_allT = sm.tile([48, H, 48], BF16, tag="p_allT", name="p_allT")
            nc.scalar.activation(
                p_allT[:], s.pop("s_ps")[:],
                mybir.ActivationFunctionType.Exp, bias=0.0, scale=SCALE)
            s["p_allT"] = p_allT
            bd = sm.tile([128, 3, C], BF16, tag="bd", name=f"bd{b}")
            nc.vector.memset(bd[:], 0.0)
            s["bd"] = bd

        def b_s5b(b):
            s = st[b]
            z_ps = bpool(b).tile([1, C], F32, tag=btag(b), name="z_ps")
            nc.tensor.matmul(z_ps[:], ones48[:], s["p_allT"][:],
                             start=True, stop=True)
            zr = sm.tile([1, C], F32, tag="zr", name="zr")
            nc.vector.reciprocal(zr[:], z_ps[:])
            s["zr"] = zr

        def b_s5c(b):
            s = st[b]
            p_allT = s["p_allT"]
            zb_ps = bpool(b).tile([48, H, 48], F32, tag=btag(b), name="zb_ps")
            nc.tensor.matmul(zb_ps[:], ones_r[:], s["zr"][:],
                             start=True, stop=True)
            attn_t = sm.tile([48, H, 48], BF16, tag="attn_t", name="attn_t")
            nc.vector.tensor_mul(attn_t[:], p_allT[:], zb_ps[:])
            s["attn_t"] = attn_t

        def b_s6(b):
            s = st[b]
            attn_t, bd = s["attn_t"], s["bd"]
            for kc in range(3):
                bd_ps = bpool(b).tile([128, C], F32, tag=btag(b), name="bd_ps")
                for i, (kc_i, h, off) in enumerate(SCATTER):
                    if kc_i != kc:
                        continue
                    nc.tensor.matmul(
                        bd_ps[:, 48 * h:48 * h + 48],
                        sh_sb[:, i, :], attn_t[:, h, :],
                        start=True, stop=True)
                lo, hi = BD_COLS[kc]
                nc.scalar.activation(
                    bd[:, kc, lo:hi], bd_ps[:, lo:hi],
                    mybir.ActivationFunctionType.Identity, bias=0.0, scale=1.0)

        # ---- phase C: stage5 (attn @ v via block-diag) + stage6 (out proj)
        def c_start(b):
            s = st[b]
            s["at"] = big1.tile([128, C * NCHUNK], F32R, tag="at", name=f"at{b}")
            s["atv"] = s["at"][:].rearrange("p (d h t) -> p h d t", h=H, t=NCHUNK)
            s["atr"] = s["at"][:].rearrange("p (r j) -> p r j", j=3)

        def c_chunk(b, t):
            s = st[b]
            vT, bd = s["vT"], s["bd"]
            pool, tg = (ps_c, "psc") if t % 2 == 0 else (ps_v, "vps")
            at_ps = pool.tile([128, C], F32, tag=tg, name="at_ps")
            for kc in range(3):
                nc.tensor.matmul(
                    at_ps[:],
                    vT[:, kc, t * 128:(t + 1) * 128],
                    bd[:, kc, :],
                    start=(kc == 0), stop=(kc == 2))
            nc.scalar.activation(
                s["atv"][:, 0:5, :, t], at_ps[:, 0:240],
                mybir.ActivationFunctionType.Identity, bias=0.0, scale=1.0)
            nc.vector.tensor_copy(s["atv"][:, 5:8, :, t], at_ps[:, 240:384])

        def c_out(b, rw, blk, last=False):
            s = st[b]
            if s.get("y_blk") is None:
                s["y_sb"] = yo.tile([128, blk, C], BF16, tag="ysb", name="y_sb")
                s["y_blk"], s["y_lo"] = blk, rw
            y_ps = ps_g.tile([128, C], F32, tag="g", name="y_ps")
            for j in range(3):
                nc.tensor.matmul(
                    y_ps[:],
                    s["atr"][:, rw * 128:(rw + 1) * 128, j],
                    wp_sb[:, j, :],
                    start=(j == 0), stop=(j == 2))
            nc.vector.tensor_add(s["y_sb"][:, rw - s["y_lo"], :], y_ps[:],
                                 bp_bc[:])
            if rw - s["y_lo"] == s["y_blk"] - 1:
                dst = outp.ap()[b].rearrange("(t p) c -> p t c", p=128)
                dst = dst[:, s["y_lo"]:rw + 1, :]
                if last:
                    # final writeback on the critical path: HWDGE queues
                    # (ACT + SP are idle by now), split across two queues
                    nc.scalar.dma_start(dst[0:64], s["y_sb"][0:64])
                    nc.sync.dma_start(dst[64:128], s["y_sb"][64:128])
                else:
                    nc.sync.dma_start(dst, s["y_sb"][:])
                s["y_blk"] = None

        def c_out_last(b, rw):
            # single-chunk writeback pieces on the two HWDGE queues so the
            # final data drains as it is produced instead of in a burst
            s = st[b]
            y_sb = yo.tile([128, 1, C], BF16, tag="ysb", name="y_sb")
            y_ps = ps_g.tile([128, C], F32, tag="g", name="y_ps")
            for j in range(3):
                nc.tensor.matmul(
                    y_ps[:],
                    s["atr"][:, rw * 128:(rw + 1) * 128, j],
                    wp_sb[:, j, :],
                    start=(j == 0), stop=(j == 2))
            nc.vector.tensor_add(y_sb[:, 0, :], y_ps[:], bp_bc[:])
            dst = outp.ap()[b].rearrange("(t p) c -> p t c", p=128)
            dst = dst[:, rw:rw + 1, :]
            eng = nc.scalar if rw % 2 == 0 else nc.sync
            eng.dma_start(dst[0:64], y_sb[0:64])
            eng2 = nc.sync if rw % 2 == 0 else nc.scalar
            eng2.dma_start(dst[64:128], y_sb[64:128])

        # ---- schedule ------------------------------------------------------
        a_step(0, 0)
        a_step(0, 1)
        load_qk()
        a_step(0, 2)
        a_step(0, 3)
        load_p()
        for w in range(4, NWIN):
            a_step(0, w)
        a_tail(0)

        b_s1_act(0)
        a_step(1, 0)
        b_s1_pe(0)
        a_step(1, 1)
        b_s2(0)
        a_step(1, 2)
        b_s34(0, range(0, 4))
        a_step(1, 3)
        b_s34(0, range(4, 8))
        b_s5a(0)
        a_step(1, 4)
        b_s5b(0)
        a_step(1, 5)
        b_s5c(0)
        a_step(1, 6)
        b_s6(0)
        a_step(1, 7)
        a_tail(1)

        c_start(0)
        b1_stages = [lambda: b_s1_act(1), lambda: b_s1_pe(1),
                     lambda: b_s2(1),
                     lambda: b_s34(1, range(0, 4)),
                     lambda: (b_s34(1, range(4, 8)), b_s5a(1)),
                     lambda: b_s5b(1),
                     lambda: b_s5c(1), lambda: b_s6(1)]
        for t in range(NCHUNK):
            c_chunk(0, t)
            if t % 2 == 1 and t // 2 < len(b1_stages):
                b1_stages[t // 2]()
        for rw in range(NCHUNK):
            c_out(0, rw, 8)
        c_start(1)
        for t in range(NCHUNK):
            c_chunk(1, t)
        for lo, blk in [(0, 4), (4, 4), (8, 4), (12, 4), (16, 4), (20, 4)]:
            for rw in range(lo, lo + blk):
                c_out(1, rw, blk)
        for rw in range(24, NCHUNK):
            c_out_last(1, rw)

    nc.compile()
    return nc


_CACHE = {}


def prepare_in_maps(x, conv_w, conv_b, wq, bq, wkv, bkv, wp, bp):
    import ml_dtypes

    bf16 = ml_dtypes.bfloat16
    f8 = ml_dtypes.float8_e4m3
    f32 = np.float32
    x = np.ascontiguousarray(x, dtype=f32)

    # fold the 1x1 conv into the projections (host-side weight prep)
    wk_w, wv_w = wkv[:C], wkv[C:]
    bk_b, bv_b = bkv[:C], bkv[C:]
    wqT = np.ascontiguousarray((wq @ conv_w).T, dtype=bf16)
    wkT = np.ascontiguousarray((wk_w @ conv_w).T, dtype=bf16)
    wvT = np.ascontiguousarray((wv_w @ conv_w).T, dtype=bf16)
    wpT = np.ascontiguousarray(wp.T, dtype=f32)
    bq_e = (bq + wq @ conv_b).astype(f32)
    bk_e = (bk_b + wk_w @ conv_b).astype(f32)
    bv_e = np.ascontiguousarray(bv_b + wv_w @ conv_b, dtype=f32)
    bp_c = np.ascontiguousarray(bp, dtype=f32)

    # partition-major chunked layouts: each SBUF partition's window data is
    # contiguous in DRAM, so window DMAs are 128 large descriptors instead
    # of 384-512 small ones (descriptor generation gates arrival latency).
    xt = x.transpose(0, 2, 1).astype(bf16)               # [B, C, N]
    xt = np.ascontiguousarray(
        xt.reshape(B, 3, 128, NWIN, NW).transpose(0, 2, 3, 1, 4))
    xn = np.zeros((B, N, XNP), dtype=f8)
    xn[:, :, :C] = x.astype(f8)
    xn[:, :, C] = 1.0
    xn = np.ascontiguousarray(
        xn.reshape(B, NCHUNK, 128, XNP).transpose(0, 2, 1, 3))

    bq_bf = np.ascontiguousarray(bq_e.reshape(1, C), dtype=bf16)
    bk_bf = np.ascontiguousarray(bk_e.reshape(1, C), dtype=bf16)
    bqn = np.ascontiguousarray((bq_e * N).reshape(1, C), dtype=f32)

    in_maps = []
    for c in range(N_CORES):
        in_maps.append({
            "x": xt[c * BPC:(c + 1) * BPC],
            "xn": xn[c * BPC:(c + 1) * BPC],
            "wqT": wqT, "wkT": wkT, "wvT": wvT, "wpT": wpT,
            "bq": bq_bf, "bk": bk_bf, "bqN": bqn, "bv": bv_e, "bp": bp_c,
        })

    return in_maps


def kernel(x, conv_w, conv_b, wq, bq, wkv, bkv, wp, bp):
    _install_ntff_hook()
    in_maps = prepare_in_maps(x, conv_w, conv_b, wq, bq, wkv, bkv, wp, bp)
    if "nc" not in _CACHE:
        _CACHE["nc"] = build()
    nc = _CACHE["nc"]
    res = run_bass_kernel_spmd(nc, in_maps, core_ids=list(range(N_CORES)))
    out = np.concatenate([res.results[c]["out"] for c in range(N_CORES)], axis=0)
    return out.astype(np.float32)

